# revision 5
# baseline (speedup 1.0000x reference)
"""Distributed Trainium2 kernel for nn_AdaptivePooling (sliding-window
mean/max/logvar pooling + linear projection).

Reference computation (B=64, D=256, T=4096, kernel=16, stride=8, N=511):
    win[b,d,n,:] = x[b, d, 8n : 8n+16]
    pooled = w0*mean(win) + w1*max(win) + w2*log(clip(var_unbiased(win)))
    out[b,e,n] = sum_d proj_w[e,d] * pooled[b,d,n] + proj_b[e]
with [w0,w1,w2] = softmax(pool_weights).

Strategy: data-parallel over batch across 8 NeuronCores (8 batches/core).

Numerical core ("assumed mean" one-pass): per 8-chunk c, shift by the
chunk's first element a = x[8c]:
    Y_j   = x_{8c+j} - a          (j=1..7; Y_0 = 0 by construction)
    S1    = sum_j Y_j             (TensorE, fp16 identity matmul, f32 PSUM)
    SSQ   = sum_j Y_j^2           (TensorE, bf16 identity matmul)
    M2c8  = SSQ - S1^2/8          (shift-invariant; since (a-mean)^2 <= M2,
                                   the cancellation error is always relative)
    mean8 = a + S1/8,  max8 = a + max(0, max_j Y_j)
Window (16) stats via Chan:  q = M2c8[n] + M2c8[n+1] + 4*(mean8[n]-mean8[n+1])^2
    log_var = Ln(clip(q)/15) via Relu/Ln bias trick on ScalarE.

Engine balance (the kernel is elementwise-bound, not matmul-bound).
Pool (GPSIMD) only supports tensor_tensor add/sub/mult (no max, no stt),
so the max path stays on VectorE:
  VectorE: most of Y subtract, max-tree, mean8 (stt from PSUM), part of Y^2
  GPSIMD:  tail of Y subtract, window adds/subs (d, m2c16, mean16)
  ScalarE: a-pack copy, S1^2/8 (Square from PSUM), part of Y^2, M2 copy,
           4d^2, clip(Relu)+Ln, out bias
  TensorE: S1/SSQ chunk reductions, S1^2/8 subtract, projection
"""

import numpy as np

B, D, T = 64, 256, 4096
KER, STR = 16, 8
N = (T - KER) // STR + 1  # 511
C = T // STR  # 512 chunks
N_CORES = 8
BL = B // N_CORES  # 8 batches per core

_CACHE: dict = {}


def _build(reps=1, y_pool_j=2, ysq_dve_j=2, mean8_eng="dve",
           d_eng="gpsimd", mean16_eng="gpsimd", m2c16_eng="gpsimd",
           q_eng="dve", tsub_pe=True, x_bufs=2):
    from concourse import bacc, mybir, tile

    F32 = mybir.dt.float32
    BF16 = mybir.dt.bfloat16
    FP16 = mybir.dt.float16
    ALU = mybir.AluOpType
    ACT = mybir.ActivationFunctionType

    nc = bacc.Bacc("TRN2", target_bir_lowering=False, debug=False,
                   num_devices=N_CORES)
    x_ext = nc.dram_tensor("x", [BL, D, T], F32, kind="ExternalInput").ap()
    wt_ext = nc.dram_tensor("wt", [128, 6, 256], BF16, kind="ExternalInput").ap()
    beff_ext = nc.dram_tensor("beff", [128, 2], F32, kind="ExternalInput").ap()
    eyeh_ext = nc.dram_tensor("eyeh", [128, 128], FP16, kind="ExternalInput").ap()
    eyeb_ext = nc.dram_tensor("eyeb", [128, 128], BF16, kind="ExternalInput").ap()
    eyen_ext = nc.dram_tensor("eyen", [128, 128], F32, kind="ExternalInput").ap()
    out_ext = nc.dram_tensor("out", [BL, D, N], F32, kind="ExternalOutput").ap()

    def eng(name):
        return {"gpsimd": nc.gpsimd, "dve": nc.vector}[name]

    with tile.TileContext(nc) as tc:
        with (
            tc.tile_pool(name="wpool", bufs=1) as wpool,
            tc.tile_pool(name="xpool", bufs=x_bufs) as xpool,
            tc.tile_pool(name="ypool", bufs=2) as ypool,
            tc.tile_pool(name="ysqp", bufs=1) as ysqp,
            tc.tile_pool(name="l1p", bufs=1) as l1p,
            tc.tile_pool(name="l2p", bufs=1) as l2p,
            tc.tile_pool(name="smallp", bufs=2) as smallp,
            tc.tile_pool(name="winp", bufs=2) as winp,
            tc.tile_pool(name="stp", bufs=2) as stp,
            tc.tile_pool(name="opool", bufs=2) as opool,
            tc.tile_pool(name="ps_s1", bufs=2, space="PSUM") as ps_s1p,
            tc.tile_pool(name="ps_m2", bufs=1, space="PSUM") as ps_m2p,
            tc.tile_pool(name="ps_o", bufs=2, space="PSUM") as ps_op,
        ):
            wt = wpool.tile([128, 6, 256], BF16)
            nc.sync.dma_start(wt[:], wt_ext[:])
            beff = wpool.tile([128, 2], F32)
            nc.sync.dma_start(beff[:], beff_ext[:])
            eyeh = wpool.tile([128, 128], FP16)
            nc.sync.dma_start(eyeh[:], eyeh_ext[:])
            eyeb = wpool.tile([128, 128], BF16)
            nc.sync.dma_start(eyeb[:], eyeb_ext[:])
            eyen = wpool.tile([128, 128], F32)
            nc.sync.dma_start(eyen[:], eyen_ext[:])
            clip_lo = wpool.tile([128, 1], F32)
            nc.vector.memset(clip_lo[:], -15e-6)
            ln_bias = wpool.tile([128, 1], F32)
            nc.vector.memset(ln_bias[:], 1e-6)

            rep_ctx = tc.For_i(0, reps, 1) if reps > 1 else None
            if rep_ctx is not None:
                rep_ctx.__enter__()
            for b in range(BL):
                X = xpool.tile([128, 2, T], F32, tag="x")
                nc.sync.dma_start(
                    X[:], x_ext[b].rearrange("(h p) t -> p h t", p=128))
                X4 = X[:].rearrange("p h (c k) -> p h c k", k=8)
                a_sl = X4[:, :, :, 0]  # [128, 2, C] stride-8 view

                # --- dense copy of the per-chunk shifts (ScalarE) ---
                apack = smallp.tile([128, 2, C], F32, tag="apack")
                nc.scalar.copy(apack[:], a_sl)

                # --- Y = x - a (j=1..7), fp16; split VectorE / Pool ---
                Yt = ypool.tile([128, 2, C, 7], FP16, tag="y")
                yp = y_pool_j
                ys = 7 - yp
                if ys > 0:
                    ab = a_sl.rearrange("p h (c o) -> p h c o", o=1) \
                             .broadcast_to([128, 2, C, ys])
                    nc.vector.tensor_tensor(
                        Yt[:, :, :, 0:ys], X4[:, :, :, 1:1 + ys], ab,
                        op=ALU.subtract)
                if yp > 0:
                    abp = a_sl.rearrange("p h (c o) -> p h c o", o=1) \
                              .broadcast_to([128, 2, C, yp])
                    nc.gpsimd.tensor_tensor(
                        Yt[:, :, :, ys:7], X4[:, :, :, 1 + ys:8], abp,
                        op=ALU.subtract)

                # --- S1 = sum_j Y_j on TensorE (fp16 identity) ---
                ps_s1 = ps_s1p.tile([128, 2, C], F32, tag="ps_s1")
                for h in range(2):
                    for jj in range(7):
                        nc.tensor.matmul(ps_s1[:, h, :], eyeh[:],
                                         Yt[:, h, :, jj],
                                         start=(jj == 0), stop=(jj == 6))
                # t8 = S1^2/8 (ScalarE Square, straight from PSUM)
                t8 = smallp.tile([128, 2, C], F32, tag="t8")
                nc.scalar.activation(t8[:], ps_s1[:], ACT.Square,
                                     scale=float(1.0 / np.sqrt(8.0)))

                # --- mean8 = a + S1/8 (stt; in0 from PSUM) ---
                mean8 = smallp.tile([128, 2, C], F32, tag="mean8")
                eng(mean8_eng).scalar_tensor_tensor(
                    mean8[:], ps_s1[:], 0.125, apack[:],
                    op0=ALU.mult, op1=ALU.add)

                # --- max tree from Y (fp16) ---
                # mdev = max over j=1..7 of Y_j ; pairs (0,3)(1,4)(2,5)(3,6)
                L1 = l1p.tile([128, 2, C, 4], FP16, tag="l1")
                nc.vector.tensor_tensor(L1[:], Yt[:, :, :, 0:4],
                                        Yt[:, :, :, 3:7], op=ALU.max)
                L2 = l2p.tile([128, 2, C, 2], FP16, tag="l2")
                nc.vector.tensor_tensor(L2[:], L1[:, :, :, 0:2],
                                        L1[:, :, :, 2:4], op=ALU.max)
                mdev = smallp.tile([128, 2, C], FP16, tag="mdev")
                nc.vector.tensor_tensor(mdev[:], L2[:, :, :, 0],
                                        L2[:, :, :, 1], op=ALU.max)
                # max8 = max(mdev, 0) + a
                max8 = smallp.tile([128, 2, C], BF16, tag="max8")
                nc.vector.scalar_tensor_tensor(
                    max8[:], mdev[:], 0.0, apack[:], op0=ALU.max, op1=ALU.add)

                # --- YSQ = Y*Y (split VectorE / ScalarE) ---
                YSQ = ysqp.tile([128, 2, C, 7], BF16, tag="ysq")
                s = ysq_dve_j
                if s > 0:
                    nc.vector.tensor_tensor(
                        YSQ[:, :, :, 0:s], Yt[:, :, :, 0:s], Yt[:, :, :, 0:s],
                        op=ALU.mult)
                if s < 7:
                    nc.scalar.activation(YSQ[:, :, :, s:7], Yt[:, :, :, s:7],
                                         ACT.Square)

                # --- M2c8 = sum_j Y^2 - t8 on TensorE ---
                ps_m2 = ps_m2p.tile([128, 2, C], F32, tag="ps_m2")
                for h in range(2):
                    for jj in range(7):
                        nc.tensor.matmul(ps_m2[:, h, :], eyeb[:],
                                         YSQ[:, h, :, jj],
                                         start=(jj == 0), stop=False)
                    if tsub_pe:
                        nc.tensor.matmul(ps_m2[:, h, :], eyen[:],
                                         t8[:, h, :], start=False, stop=True)
                    else:
                        nc.tensor.matmul(ps_m2[:, h, :], eyeb[:],
                                         YSQ[:, h, :, 6],
                                         start=False, stop=True)
                m2c8 = smallp.tile([128, 2, C], F32, tag="m2c8")
                nc.scalar.copy(m2c8[:], ps_m2[:])
                if not tsub_pe:
                    # fallback: subtract t8 on DVE (also fix double-added j6)
                    nc.vector.tensor_sub(m2c8[:], m2c8[:], t8[:])

                # --- window stats ---
                st = stp.tile([128, 2, 3, N], BF16, tag="st")
                # max16
                nc.vector.tensor_tensor(
                    st[:, :, 1, :], max8[:, :, 0:N], max8[:, :, 1:C],
                    op=ALU.max)
                # mean16 -> st0 (bf16), scale 0.5 folded into host weights
                eng(mean16_eng).tensor_tensor(
                    st[:, :, 0, :], mean8[:, :, 0:N], mean8[:, :, 1:C],
                    op=ALU.add)
                # m2c16
                m2c16 = winp.tile([128, 2, N], F32, tag="m2c16")
                eng(m2c16_eng).tensor_tensor(
                    m2c16[:], m2c8[:, :, 0:N], m2c8[:, :, 1:C], op=ALU.add)
                # d = mean8[n] - mean8[n+1]; dq = 4*d^2 (in place over d)
                dwin = winp.tile([128, 2, N], F32, tag="dwin")
                eng(d_eng).tensor_tensor(
                    dwin[:], mean8[:, :, 0:N], mean8[:, :, 1:C],
                    op=ALU.subtract)
                nc.scalar.activation(dwin[:], dwin[:], ACT.Square, scale=2.0)
                # q = m2c16 + dq   (in place over m2c16)
                eng(q_eng).tensor_tensor(
                    m2c16[:], m2c16[:], dwin[:], op=ALU.add)
                # clip + Ln:  ln(max(q,15e-6)/15) via Relu/Ln bias trick
                nc.scalar.activation(m2c16[:], m2c16[:], ACT.Relu,
                                     bias=clip_lo[:])
                nc.scalar.activation(st[:, :, 2, :], m2c16[:], ACT.Ln,
                                     scale=1.0 / 15.0, bias=ln_bias[:])

                # --- projection ---
                for eh in range(2):
                    ps = ps_op.tile([128, N], F32, tag="ps_o")
                    k = 0
                    for s3 in range(3):
                        for h in range(2):
                            nc.tensor.matmul(
                                ps[:],
                                wt[:, s3 * 2 + h, eh * 128:(eh + 1) * 128],
                                st[:, h, s3, :],
                                start=(k == 0), stop=(k == 5))
                            k += 1
                    ob = opool.tile([128, N], F32, tag="ob")
                    nc.scalar.activation(ob[:], ps[:], ACT.Identity,
                                         bias=beff[:, eh:eh + 1], scale=1.0)
                    nc.sync.dma_start(out_ext[b, eh * 128:(eh + 1) * 128, :],
                                      ob[:])

            if rep_ctx is not None:
                rep_ctx.__exit__(None, None, None)

    nc.compile()
    return nc


def _get_nc():
    if "nc" not in _CACHE:
        _CACHE["nc"] = _build()
    return _CACHE["nc"]


def _prep_host(pool_weights, proj_w, proj_b):
    from concourse import mybir
    BF16_NP = mybir.dt.np(mybir.dt.bfloat16)
    FP16_NP = np.float16

    pw = np.asarray(pool_weights, np.float32)
    e = np.exp(pw - pw.max())
    w = (e / e.sum()).astype(np.float32)

    W = np.asarray(proj_w, np.float32)  # [E, D]
    # st0 = mean8[n]+mean8[n+1] = 2*mean16  ->  weight w0/2
    Wcat = np.concatenate(
        [(w[0] / 2.0) * W, w[1] * W, w[2] * W], axis=1)  # [256, 768]
    lhsT = np.ascontiguousarray(Wcat.T)  # [768, 256]
    wt_host = np.ascontiguousarray(
        lhsT.reshape(6, 128, 256).transpose(1, 0, 2)).astype(BF16_NP)
    beff_host = np.ascontiguousarray(
        np.asarray(proj_b, np.float32).reshape(2, 128).T)
    eyeh = np.eye(128, dtype=np.float32).astype(FP16_NP)
    eyeb = np.eye(128, dtype=np.float32).astype(BF16_NP)
    eyen = (-np.eye(128, dtype=np.float32))
    return wt_host, beff_host, eyeh, eyeb, eyen


def _get_runner():
    """Cached jitted SPMD runner (avoids re-tracing the PJRT wrapper on
    every kernel() call).  Mirrors bass2jax.run_bass_via_pjrt."""
    if "runner" in _CACHE:
        return _CACHE["runner"]

    import jax
    from concourse import mybir
    from concourse.bass2jax import (
        _bass_exec_p, install_neuronx_cc_hook, partition_id_tensor)
    from jax.sharding import Mesh, PartitionSpec
    from jax.experimental.shard_map import shard_map

    nc = _get_nc()
    install_neuronx_cc_hook()

    partition_name = (nc.partition_id_tensor.name
                      if nc.partition_id_tensor else None)
    in_names, out_names, out_avals, zero_shapes = [], [], [], []
    for alloc in nc.m.functions[0].allocations:
        if not isinstance(alloc, mybir.MemoryLocationSet):
            continue
        name = alloc.memorylocations[0].name
        if alloc.kind == "ExternalInput":
            if name != partition_name:
                in_names.append(name)
        elif alloc.kind == "ExternalOutput":
            out_names.append(name)
            shape = tuple(alloc.tensor_shape)
            dtype = mybir.dt.np(alloc.dtype)
            out_avals.append(jax.core.ShapedArray(shape, dtype))
            zero_shapes.append((shape, dtype))
    n_params = len(in_names)
    n_outs = len(out_avals)
    all_in = in_names + out_names + ([partition_name] if partition_name else [])

    def _body(*args):
        operands = list(args)
        if partition_name is not None:
            operands.append(partition_id_tensor())
        outs = _bass_exec_p.bind(
            *operands, out_avals=tuple(out_avals), in_names=tuple(all_in),
            out_names=tuple(out_names), lowering_input_output_aliases=(),
            sim_require_finite=True, sim_require_nnan=True, nc=nc)
        return tuple(outs)

    devices = jax.devices()[:N_CORES]
    mesh = Mesh(np.asarray(devices), ("core",))
    in_specs = (PartitionSpec("core"),) * (n_params + n_outs)
    out_specs = (PartitionSpec("core"),) * n_outs
    donate = tuple(range(n_params, n_params + n_outs))
    sharded = jax.jit(
        shard_map(_body, mesh=mesh, in_specs=in_specs, out_specs=out_specs,
                  check_rep=False),
        donate_argnums=donate, keep_unused=True)
    sharding = jax.sharding.NamedSharding(mesh, PartitionSpec("core"))

    def run(in_maps):
        concat_in = [
            np.concatenate(
                [np.asarray(in_maps[c][nm]) for c in range(N_CORES)], axis=0)
            for nm in in_names
        ]
        dev_in = [jax.device_put(a, sharding) for a in concat_in]
        zs = [
            jax.device_put(
                np.zeros((N_CORES * s[0], *s[1:]), dt), sharding)
            for (s, dt) in zero_shapes
        ]
        outs = sharded(*dev_in, *zs)
        return {
            nm: np.asarray(outs[i]).reshape(N_CORES, *out_avals[i].shape)
            for i, nm in enumerate(out_names)
        }

    _CACHE["runner"] = run
    return run


def kernel(x, pool_weights, proj_w, proj_b):
    wt_host, beff_host, eyeh, eyeb, eyen = _prep_host(
        pool_weights, proj_w, proj_b)
    x_f = np.ascontiguousarray(np.asarray(x, np.float32))

    in_maps = [
        {"x": x_f[i * BL:(i + 1) * BL], "wt": wt_host, "beff": beff_host,
         "eyeh": eyeh, "eyeb": eyeb, "eyen": eyen}
        for i in range(N_CORES)
    ]
    res = _get_runner()(in_maps)
    out = res["out"].reshape(B, D, N)
    return np.ascontiguousarray(out.astype(np.float32))


# revision 8
# speedup vs baseline: 1.1998x; 1.1998x over previous
"""Distributed Trainium2 kernel for nn_AdaptivePooling (sliding-window
mean/max/logvar pooling + linear projection).

Reference computation (B=64, D=256, T=4096, kernel=16, stride=8, N=511):
    win[b,d,n,:] = x[b, d, 8n : 8n+16]
    pooled = w0*mean(win) + w1*max(win) + w2*log(clip(var_unbiased(win)))
    out[b,e,n] = sum_d proj_w[e,d] * pooled[b,d,n] + proj_b[e]
with [w0,w1,w2] = softmax(pool_weights).

Strategy: data-parallel over batch across 8 NeuronCores (8 batches/core).

Numerical core ("assumed mean" one-pass): per 8-chunk c, shift by the
chunk's first element a = x[8c]:
    Y_j   = x_{8c+j} - a          (j=1..7; Y_0 = 0 by construction)
    S1    = sum_j Y_j             (TensorE, fp16 identity matmul, f32 PSUM)
    SSQ   = sum_j Y_j^2           (TensorE, bf16 identity matmul)
    M2c8  = SSQ - S1^2/8          (shift-invariant; since (a-mean)^2 <= M2,
                                   the cancellation error is always relative)
    mean8 = a + S1/8,  max8 = a + max(0, max_j Y_j)
Window (16) stats via Chan:  q = M2c8[n] + M2c8[n+1] + 4*(mean8[n]-mean8[n+1])^2
    log_var = Ln(clip(q)/15) via Relu/Ln bias trick on ScalarE.

Engine balance (the kernel is elementwise-bound, not matmul-bound).
Pool (GPSIMD) only supports tensor_tensor add/sub/mult (no max, no stt),
so the max path stays on VectorE:
  VectorE: most of Y subtract, max-tree, mean8 (stt from PSUM), part of Y^2
  GPSIMD:  tail of Y subtract, window adds/subs (d, m2c16, mean16)
  ScalarE: a-pack copy, S1^2/8 (Square from PSUM), part of Y^2, M2 copy,
           4d^2, clip(Relu)+Ln, out bias
  TensorE: S1/SSQ chunk reductions, S1^2/8 subtract, projection
"""

import numpy as np

B, D, T = 64, 256, 4096
KER, STR = 16, 8
N = (T - KER) // STR + 1  # 511
C = T // STR  # 512 chunks
N_CORES = 8
BL = B // N_CORES  # 8 batches per core

_CACHE: dict = {}


def _build(reps=1, y_pool_j=2, mean8_eng="dve", d_eng="gpsimd",
           mean16_eng="gpsimd", m2c16_eng="gpsimd", q_eng="dve",
           x_bufs=2, y_bufs=4, small_bufs=4, win_bufs=4, st_bufs=4,
           l_bufs=2, ysq_inplace=True):
    from concourse import bacc, mybir, tile

    F32 = mybir.dt.float32
    BF16 = mybir.dt.bfloat16
    FP16 = mybir.dt.float16
    ALU = mybir.AluOpType
    ACT = mybir.ActivationFunctionType

    nc = bacc.Bacc("TRN2", target_bir_lowering=False, debug=False,
                   num_devices=N_CORES)
    x_ext = nc.dram_tensor("x", [BL, D, T], F32, kind="ExternalInput").ap()
    wt_ext = nc.dram_tensor("wt", [128, 6, 256], BF16, kind="ExternalInput").ap()
    beff_ext = nc.dram_tensor("beff", [128, 2], F32, kind="ExternalInput").ap()
    eyeh_ext = nc.dram_tensor("eyeh", [128, 128], FP16, kind="ExternalInput").ap()
    eyeb_ext = nc.dram_tensor("eyeb", [128, 128], BF16, kind="ExternalInput").ap()
    eyen_ext = nc.dram_tensor("eyen", [128, 128], F32, kind="ExternalInput").ap()
    out_ext = nc.dram_tensor("out", [BL, D, N], F32, kind="ExternalOutput").ap()

    def eng(name):
        return {"gpsimd": nc.gpsimd, "dve": nc.vector}[name]

    with tile.TileContext(nc) as tc:
        with (
            tc.tile_pool(name="wpool", bufs=1) as wpool,
            tc.tile_pool(name="xpool", bufs=x_bufs) as xpool,
            tc.tile_pool(name="ypool", bufs=y_bufs) as ypool,
            tc.tile_pool(name="ysqp", bufs=2) as ysqp,
            tc.tile_pool(name="l1p", bufs=l_bufs) as l1p,
            tc.tile_pool(name="l2p", bufs=l_bufs) as l2p,
            tc.tile_pool(name="smallp", bufs=small_bufs) as smallp,
            tc.tile_pool(name="winp", bufs=win_bufs) as winp,
            tc.tile_pool(name="stp", bufs=st_bufs) as stp,
            tc.tile_pool(name="opool", bufs=2) as opool,
            tc.tile_pool(name="ps_s1", bufs=3, space="PSUM") as ps_s1p,
            tc.tile_pool(name="ps_m2", bufs=3, space="PSUM") as ps_m2p,
            tc.tile_pool(name="ps_o", bufs=2, space="PSUM") as ps_op,
        ):
            wt = wpool.tile([128, 6, 256], BF16)
            nc.sync.dma_start(wt[:], wt_ext[:])
            beff = wpool.tile([128, 2], F32)
            nc.sync.dma_start(beff[:], beff_ext[:])
            eyeh = wpool.tile([128, 128], FP16)
            nc.sync.dma_start(eyeh[:], eyeh_ext[:])
            eyeb = wpool.tile([128, 128], BF16)
            nc.sync.dma_start(eyeb[:], eyeb_ext[:])
            eyen = wpool.tile([128, 128], F32)
            nc.sync.dma_start(eyen[:], eyen_ext[:])
            clip_lo = wpool.tile([128, 1], F32)
            nc.vector.memset(clip_lo[:], -15e-6)
            ln_bias = wpool.tile([128, 1], F32)
            nc.vector.memset(ln_bias[:], 1e-6)

            rep_ctx = tc.For_i(0, reps, 1) if reps > 1 else None
            if rep_ctx is not None:
                rep_ctx.__enter__()
            for b in range(BL):
                X = xpool.tile([128, 2, T], F32, tag="x")
                nc.sync.dma_start(
                    X[:], x_ext[b].rearrange("(h p) t -> p h t", p=128))
                sts = []
                for h in range(2):
                    X4 = X[:, h].rearrange("p (c k) -> p c k", k=8)
                    a_sl = X4[:, :, 0]  # [128, C] stride-8 view

                    # dense copy of the per-chunk shifts (ScalarE)
                    apack = smallp.tile([128, C], F32, tag="apack")
                    nc.scalar.copy(apack[:], a_sl)

                    # Y = x - a (j=1..7), fp16; split VectorE / Pool
                    Yt = ypool.tile([128, C, 7], FP16, tag="y")
                    yp = y_pool_j
                    ys = 7 - yp
                    if ys > 0:
                        ab = a_sl.rearrange("p (c o) -> p c o", o=1) \
                                 .broadcast_to([128, C, ys])
                        nc.vector.tensor_tensor(
                            Yt[:, :, 0:ys], X4[:, :, 1:1 + ys], ab,
                            op=ALU.subtract)
                    if yp > 0:
                        abp = a_sl.rearrange("p (c o) -> p c o", o=1) \
                                  .broadcast_to([128, C, yp])
                        nc.gpsimd.tensor_tensor(
                            Yt[:, :, ys:7], X4[:, :, 1 + ys:8], abp,
                            op=ALU.subtract)

                    # S1 = sum_j Y_j on TensorE (fp16 identity)
                    ps_s1 = ps_s1p.tile([128, C], F32, tag="ps_s1")
                    for jj in range(7):
                        nc.tensor.matmul(ps_s1[:], eyeh[:], Yt[:, :, jj],
                                         start=(jj == 0), stop=(jj == 6))
                    # t8 = S1^2/8 (ScalarE Square, straight from PSUM)
                    t8 = smallp.tile([128, C], F32, tag="t8")
                    nc.scalar.activation(t8[:], ps_s1[:], ACT.Square,
                                         scale=float(1.0 / np.sqrt(8.0)))

                    # mean8 = a + S1/8 (stt; in0 from PSUM)
                    mean8 = smallp.tile([128, C], F32, tag="mean8")
                    eng(mean8_eng).scalar_tensor_tensor(
                        mean8[:], ps_s1[:], 0.125, apack[:],
                        op0=ALU.mult, op1=ALU.add)

                    # max tree: mdev = max_{j=1..7} Y_j
                    L1 = l1p.tile([128, C, 4], FP16, tag="l1")
                    nc.vector.tensor_tensor(L1[:], Yt[:, :, 0:4],
                                            Yt[:, :, 3:7], op=ALU.max)
                    L2 = l2p.tile([128, C, 2], FP16, tag="l2")
                    nc.vector.tensor_tensor(L2[:], L1[:, :, 0:2],
                                            L1[:, :, 2:4], op=ALU.max)
                    mdev = smallp.tile([128, C], FP16, tag="mdev")
                    nc.vector.tensor_tensor(mdev[:], L2[:, :, 0],
                                            L2[:, :, 1], op=ALU.max)
                    # max8 = max(mdev, 0) + a
                    max8 = smallp.tile([128, C], BF16, tag="max8")
                    nc.vector.scalar_tensor_tensor(
                        max8[:], mdev[:], 0.0, apack[:],
                        op0=ALU.max, op1=ALU.add)

                    # YSQ = Y*Y on ScalarE (in place over Yt, bf16 view)
                    if ysq_inplace:
                        YSQ = Yt[:].bitcast(BF16)
                    else:
                        ysq_t = ysqp.tile([128, C, 7], BF16, tag="ysq")
                        YSQ = ysq_t[:]
                    nc.scalar.activation(YSQ, Yt[:], ACT.Square)

                    # M2c8 = sum_j Y^2 (TensorE) - t8 (DVE, PSUM read)
                    ps_m2 = ps_m2p.tile([128, C], F32, tag="ps_m2")
                    for jj in range(7):
                        nc.tensor.matmul(ps_m2[:], eyeb[:], YSQ[:, :, jj],
                                         start=(jj == 0), stop=(jj == 6))
                    m2c8 = smallp.tile([128, C], F32, tag="m2c8")
                    nc.vector.tensor_sub(m2c8[:], ps_m2[:], t8[:])

                    # window stats
                    st = stp.tile([128, 3, N], BF16, tag="st")
                    nc.vector.tensor_tensor(
                        st[:, 1, :], max8[:, 0:N], max8[:, 1:C], op=ALU.max)
                    eng(mean16_eng).tensor_tensor(
                        st[:, 0, :], mean8[:, 0:N], mean8[:, 1:C],
                        op=ALU.add)
                    m2c16 = winp.tile([128, N], F32, tag="m2c16")
                    eng(m2c16_eng).tensor_tensor(
                        m2c16[:], m2c8[:, 0:N], m2c8[:, 1:C], op=ALU.add)
                    dwin = winp.tile([128, N], F32, tag="dwin")
                    eng(d_eng).tensor_tensor(
                        dwin[:], mean8[:, 0:N], mean8[:, 1:C],
                        op=ALU.subtract)
                    nc.scalar.activation(dwin[:], dwin[:], ACT.Square,
                                         scale=2.0)
                    eng(q_eng).tensor_tensor(
                        m2c16[:], m2c16[:], dwin[:], op=ALU.add)
                    nc.scalar.activation(m2c16[:], m2c16[:], ACT.Relu,
                                         bias=clip_lo[:])
                    nc.scalar.activation(st[:, 2, :], m2c16[:], ACT.Ln,
                                         scale=1.0 / 15.0, bias=ln_bias[:])
                    sts.append(st)

                # projection (contracts d across both halves)
                for eh in range(2):
                    ps = ps_op.tile([128, N], F32, tag="ps_o")
                    k = 0
                    for s3 in range(3):
                        for h in range(2):
                            nc.tensor.matmul(
                                ps[:],
                                wt[:, s3 * 2 + h, eh * 128:(eh + 1) * 128],
                                sts[h][:, s3, :],
                                start=(k == 0), stop=(k == 5))
                            k += 1
                    ob = opool.tile([128, N], F32, tag="ob")
                    nc.scalar.activation(ob[:], ps[:], ACT.Identity,
                                         bias=beff[:, eh:eh + 1], scale=1.0)
                    nc.sync.dma_start(out_ext[b, eh * 128:(eh + 1) * 128, :],
                                      ob[:])

            if rep_ctx is not None:
                rep_ctx.__exit__(None, None, None)

    nc.compile()
    return nc


def _get_nc():
    if "nc" not in _CACHE:
        _CACHE["nc"] = _build()
    return _CACHE["nc"]


def _prep_host(pool_weights, proj_w, proj_b):
    from concourse import mybir
    BF16_NP = mybir.dt.np(mybir.dt.bfloat16)
    FP16_NP = np.float16

    pw = np.asarray(pool_weights, np.float32)
    e = np.exp(pw - pw.max())
    w = (e / e.sum()).astype(np.float32)

    W = np.asarray(proj_w, np.float32)  # [E, D]
    # st0 = mean8[n]+mean8[n+1] = 2*mean16  ->  weight w0/2
    Wcat = np.concatenate(
        [(w[0] / 2.0) * W, w[1] * W, w[2] * W], axis=1)  # [256, 768]
    lhsT = np.ascontiguousarray(Wcat.T)  # [768, 256]
    wt_host = np.ascontiguousarray(
        lhsT.reshape(6, 128, 256).transpose(1, 0, 2)).astype(BF16_NP)
    beff_host = np.ascontiguousarray(
        np.asarray(proj_b, np.float32).reshape(2, 128).T)
    eyeh = np.eye(128, dtype=np.float32).astype(FP16_NP)
    eyeb = np.eye(128, dtype=np.float32).astype(BF16_NP)
    eyen = (-np.eye(128, dtype=np.float32))
    return wt_host, beff_host, eyeh, eyeb, eyen


def _get_runner():
    """Cached jitted SPMD runner (avoids re-tracing the PJRT wrapper on
    every kernel() call).  Mirrors bass2jax.run_bass_via_pjrt."""
    if "runner" in _CACHE:
        return _CACHE["runner"]

    import jax
    from concourse import mybir
    from concourse.bass2jax import (
        _bass_exec_p, install_neuronx_cc_hook, partition_id_tensor)
    from jax.sharding import Mesh, PartitionSpec
    from jax.experimental.shard_map import shard_map

    nc = _get_nc()
    install_neuronx_cc_hook()

    partition_name = (nc.partition_id_tensor.name
                      if nc.partition_id_tensor else None)
    in_names, out_names, out_avals, zero_shapes = [], [], [], []
    for alloc in nc.m.functions[0].allocations:
        if not isinstance(alloc, mybir.MemoryLocationSet):
            continue
        name = alloc.memorylocations[0].name
        if alloc.kind == "ExternalInput":
            if name != partition_name:
                in_names.append(name)
        elif alloc.kind == "ExternalOutput":
            out_names.append(name)
            shape = tuple(alloc.tensor_shape)
            dtype = mybir.dt.np(alloc.dtype)
            out_avals.append(jax.core.ShapedArray(shape, dtype))
            zero_shapes.append((shape, dtype))
    n_params = len(in_names)
    n_outs = len(out_avals)
    all_in = in_names + out_names + ([partition_name] if partition_name else [])

    def _body(*args):
        operands = list(args)
        if partition_name is not None:
            operands.append(partition_id_tensor())
        outs = _bass_exec_p.bind(
            *operands, out_avals=tuple(out_avals), in_names=tuple(all_in),
            out_names=tuple(out_names), lowering_input_output_aliases=(),
            sim_require_finite=True, sim_require_nnan=True, nc=nc)
        return tuple(outs)

    devices = jax.devices()[:N_CORES]
    mesh = Mesh(np.asarray(devices), ("core",))
    in_specs = (PartitionSpec("core"),) * (n_params + n_outs)
    out_specs = (PartitionSpec("core"),) * n_outs
    donate = tuple(range(n_params, n_params + n_outs))
    sharded = jax.jit(
        shard_map(_body, mesh=mesh, in_specs=in_specs, out_specs=out_specs,
                  check_rep=False),
        donate_argnums=donate, keep_unused=True)
    sharding = jax.sharding.NamedSharding(mesh, PartitionSpec("core"))

    def run(in_maps):
        concat_in = [
            np.concatenate(
                [np.asarray(in_maps[c][nm]) for c in range(N_CORES)], axis=0)
            for nm in in_names
        ]
        dev_in = [jax.device_put(a, sharding) for a in concat_in]
        zs = [
            jax.device_put(
                np.zeros((N_CORES * s[0], *s[1:]), dt), sharding)
            for (s, dt) in zero_shapes
        ]
        outs = sharded(*dev_in, *zs)
        return {
            nm: np.asarray(outs[i]).reshape(N_CORES, *out_avals[i].shape)
            for i, nm in enumerate(out_names)
        }

    _CACHE["runner"] = run
    return run


def kernel(x, pool_weights, proj_w, proj_b):
    wt_host, beff_host, eyeh, eyeb, eyen = _prep_host(
        pool_weights, proj_w, proj_b)
    x_f = np.ascontiguousarray(np.asarray(x, np.float32))

    in_maps = [
        {"x": x_f[i * BL:(i + 1) * BL], "wt": wt_host, "beff": beff_host,
         "eyeh": eyeh, "eyeb": eyeb, "eyen": eyen}
        for i in range(N_CORES)
    ]
    res = _get_runner()(in_maps)
    out = res["out"].reshape(B, D, N)
    return np.ascontiguousarray(out.astype(np.float32))


# revision 11
# speedup vs baseline: 1.2044x; 1.0038x over previous
"""Distributed Trainium2 kernel for nn_AdaptivePooling (sliding-window
mean/max/logvar pooling + linear projection).

Reference computation (B=64, D=256, T=4096, kernel=16, stride=8, N=511):
    win[b,d,n,:] = x[b, d, 8n : 8n+16]
    pooled = w0*mean(win) + w1*max(win) + w2*log(clip(var_unbiased(win)))
    out[b,e,n] = sum_d proj_w[e,d] * pooled[b,d,n] + proj_b[e]
with [w0,w1,w2] = softmax(pool_weights).

Strategy: data-parallel over batch across 8 NeuronCores (8 batches/core),
processed at half-batch (128-channel) granularity for pipeline depth.

Numerical core ("assumed mean" one-pass): per 8-chunk c, shift by the
chunk's first element a = x[8c]:
    Y_j   = x_{8c+j} - a          (j=1..7; Y_0 = 0 by construction)
    S1    = sum_j Y_j             (TensorE, fp16 identity matmul, f32 PSUM)
    SSQ   = sum_j Y_j^2           (TensorE, bf16 identity matmul)
    M2c8  = SSQ - S1^2/8          (shift-invariant; since (a-mean)^2 <= M2,
                                   the cancellation error is always relative)
    mean8 = a + S1/8,  max8 = a + max(0, max_j Y_j)
Window (16) stats via Chan:  q = M2c8[n] + M2c8[n+1] + 4*(mean8[n]-mean8[n+1])^2
    log_var = Ln(clip(q)/15) via Relu/Ln bias trick on ScalarE.

Engine balance (the kernel is elementwise-bound, not matmul-bound).
Pool (GPSIMD) only supports tensor_tensor add/sub/mult (no max, no stt),
so the max path stays on VectorE:
  VectorE: most of Y subtract, max-tree, mean8 (stt from PSUM),
           M2 = SSQ - t8 (PSUM read), q
  GPSIMD:  tail of Y subtract, window adds/subs (d, m2c16, mean16)
  ScalarE: a-pack copy, S1^2/8 (Square from PSUM), Y^2, 4d^2,
           clip(Relu)+Ln, out bias
  TensorE: S1/SSQ chunk reductions, projection
"""

import numpy as np

B, D, T = 64, 256, 4096
KER, STR = 16, 8
N = (T - KER) // STR + 1  # 511
C = T // STR  # 512 chunks
N_CORES = 8
BL = B // N_CORES  # 8 batches per core

_CACHE: dict = {}


def _build(reps=1, y_pool_j=2, mean8_eng="dve", d_eng="gpsimd",
           mean16_eng="gpsimd", m2c16_eng="gpsimd", q_eng="dve",
           x_bufs=2, y_bufs=4, small_bufs=4, win_bufs=4, st_bufs=4,
           l_bufs=2, ysq_bufs=2, **_ignored):
    import os as _os
    import json as _json
    _env = _os.environ.get("KCFG")
    if _env:
        _o = _json.loads(_env)
        y_pool_j = _o.get("y_pool_j", y_pool_j)
        mean8_eng = _o.get("mean8_eng", mean8_eng)
        d_eng = _o.get("d_eng", d_eng)
        mean16_eng = _o.get("mean16_eng", mean16_eng)
        m2c16_eng = _o.get("m2c16_eng", m2c16_eng)
        q_eng = _o.get("q_eng", q_eng)
        x_bufs = _o.get("x_bufs", x_bufs)
        y_bufs = _o.get("y_bufs", y_bufs)
        small_bufs = _o.get("small_bufs", small_bufs)
        win_bufs = _o.get("win_bufs", win_bufs)
        st_bufs = _o.get("st_bufs", st_bufs)
        l_bufs = _o.get("l_bufs", l_bufs)
        ysq_bufs = _o.get("ysq_bufs", ysq_bufs)
    from concourse import bacc, mybir, tile

    F32 = mybir.dt.float32
    BF16 = mybir.dt.bfloat16
    FP16 = mybir.dt.float16
    ALU = mybir.AluOpType
    ACT = mybir.ActivationFunctionType

    nc = bacc.Bacc("TRN2", target_bir_lowering=False, debug=False,
                   num_devices=N_CORES)
    x_ext = nc.dram_tensor("x", [BL, D, T], F32, kind="ExternalInput").ap()
    wt_ext = nc.dram_tensor("wt", [128, 6, 256], BF16, kind="ExternalInput").ap()
    beff_ext = nc.dram_tensor("beff", [128, 2], F32, kind="ExternalInput").ap()
    eyeh_ext = nc.dram_tensor("eyeh", [128, 128], FP16, kind="ExternalInput").ap()
    eyeb_ext = nc.dram_tensor("eyeb", [128, 128], BF16, kind="ExternalInput").ap()
    eyen_ext = nc.dram_tensor("eyen", [128, 128], F32, kind="ExternalInput").ap()
    out_ext = nc.dram_tensor("out", [BL, D, N], F32, kind="ExternalOutput").ap()

    def eng(name):
        return {"gpsimd": nc.gpsimd, "dve": nc.vector}[name]

    with tile.TileContext(nc) as tc:
        with (
            tc.tile_pool(name="wpool", bufs=1) as wpool,
            tc.tile_pool(name="xpool", bufs=x_bufs) as xpool,
            tc.tile_pool(name="ypool", bufs=y_bufs) as ypool,
            tc.tile_pool(name="ysqp", bufs=ysq_bufs) as ysqp,
            tc.tile_pool(name="l1p", bufs=l_bufs) as l1p,
            tc.tile_pool(name="l2p", bufs=l_bufs) as l2p,
            tc.tile_pool(name="smallp", bufs=small_bufs) as smallp,
            tc.tile_pool(name="winp", bufs=win_bufs) as winp,
            tc.tile_pool(name="stp", bufs=st_bufs) as stp,
            tc.tile_pool(name="opool", bufs=2) as opool,
            tc.tile_pool(name="ps_s1", bufs=3, space="PSUM") as ps_s1p,
            tc.tile_pool(name="ps_m2", bufs=3, space="PSUM") as ps_m2p,
            tc.tile_pool(name="ps_o", bufs=2, space="PSUM") as ps_op,
        ):
            wt = wpool.tile([128, 6, 256], BF16)
            nc.sync.dma_start(wt[:], wt_ext[:])
            beff = wpool.tile([128, 2], F32)
            nc.sync.dma_start(beff[:], beff_ext[:])
            eyeh = wpool.tile([128, 128], FP16)
            nc.sync.dma_start(eyeh[:], eyeh_ext[:])
            eyeb = wpool.tile([128, 128], BF16)
            nc.sync.dma_start(eyeb[:], eyeb_ext[:])
            eyen = wpool.tile([128, 128], F32)
            nc.sync.dma_start(eyen[:], eyen_ext[:])
            clip_lo = wpool.tile([128, 1], F32)
            nc.vector.memset(clip_lo[:], -15e-6)
            ln_bias = wpool.tile([128, 1], F32)
            nc.vector.memset(ln_bias[:], 1e-6)

            rep_ctx = tc.For_i(0, reps, 1) if reps > 1 else None
            if rep_ctx is not None:
                rep_ctx.__enter__()
            for b in range(BL):
                X = xpool.tile([128, 2, T], F32, tag="x")
                nc.sync.dma_start(
                    X[:], x_ext[b].rearrange("(h p) t -> p h t", p=128))
                sts = []
                for h in range(2):
                    X4 = X[:, h].rearrange("p (c k) -> p c k", k=8)
                    a_sl = X4[:, :, 0]  # [128, C] stride-8 view

                    # dense copy of the per-chunk shifts (ScalarE)
                    apack = smallp.tile([128, C], F32, tag="apack")
                    nc.scalar.copy(apack[:], a_sl)

                    # Y = x - a (j=1..7), fp16; split VectorE / Pool
                    Yt = ypool.tile([128, C, 7], FP16, tag="y")
                    yp = y_pool_j
                    ys = 7 - yp
                    if ys > 0:
                        ab = a_sl.rearrange("p (c o) -> p c o", o=1) \
                                 .broadcast_to([128, C, ys])
                        nc.vector.tensor_tensor(
                            Yt[:, :, 0:ys], X4[:, :, 1:1 + ys], ab,
                            op=ALU.subtract)
                    if yp > 0:
                        abp = a_sl.rearrange("p (c o) -> p c o", o=1) \
                                  .broadcast_to([128, C, yp])
                        nc.gpsimd.tensor_tensor(
                            Yt[:, :, ys:7], X4[:, :, 1 + ys:8], abp,
                            op=ALU.subtract)

                    # S1 = sum_j Y_j on TensorE (fp16 identity)
                    ps_s1 = ps_s1p.tile([128, C], F32, tag="ps_s1")
                    for jj in range(7):
                        nc.tensor.matmul(ps_s1[:], eyeh[:], Yt[:, :, jj],
                                         start=(jj == 0), stop=(jj == 6))
                    # t8 = S1^2/8 (ScalarE Square, straight from PSUM)
                    t8 = smallp.tile([128, C], F32, tag="t8")
                    nc.scalar.activation(t8[:], ps_s1[:], ACT.Square,
                                         scale=float(1.0 / np.sqrt(8.0)))

                    # mean8 = a + S1/8 (stt; in0 from PSUM)
                    mean8 = smallp.tile([128, C], F32, tag="mean8")
                    eng(mean8_eng).scalar_tensor_tensor(
                        mean8[:], ps_s1[:], 0.125, apack[:],
                        op0=ALU.mult, op1=ALU.add)

                    # max tree: mdev = max_{j=1..7} Y_j
                    L1 = l1p.tile([128, C, 4], FP16, tag="l1")
                    nc.vector.tensor_tensor(L1[:], Yt[:, :, 0:4],
                                            Yt[:, :, 3:7], op=ALU.max)
                    L2 = l2p.tile([128, C, 2], FP16, tag="l2")
                    nc.vector.tensor_tensor(L2[:], L1[:, :, 0:2],
                                            L1[:, :, 2:4], op=ALU.max)
                    mdev = smallp.tile([128, C], FP16, tag="mdev")
                    nc.vector.tensor_tensor(mdev[:], L2[:, :, 0],
                                            L2[:, :, 1], op=ALU.max)
                    # max8 = max(mdev, 0) + a
                    max8 = smallp.tile([128, C], BF16, tag="max8")
                    nc.vector.scalar_tensor_tensor(
                        max8[:], mdev[:], 0.0, apack[:],
                        op0=ALU.max, op1=ALU.add)

                    # YSQ = Y*Y on ScalarE
                    ysq_t = ysqp.tile([128, C, 7], BF16, tag="ysq")
                    nc.scalar.activation(ysq_t[:], Yt[:], ACT.Square)

                    # M2c8 = sum_j Y^2 (TensorE) - t8 (DVE, PSUM read)
                    ps_m2 = ps_m2p.tile([128, C], F32, tag="ps_m2")
                    for jj in range(7):
                        nc.tensor.matmul(ps_m2[:], eyeb[:], ysq_t[:, :, jj],
                                         start=(jj == 0), stop=(jj == 6))
                    m2c8 = smallp.tile([128, C], F32, tag="m2c8")
                    nc.vector.tensor_sub(m2c8[:], ps_m2[:], t8[:])

                    # window stats
                    st = stp.tile([128, 3, N], BF16, tag="st")
                    nc.vector.tensor_tensor(
                        st[:, 1, :], max8[:, 0:N], max8[:, 1:C], op=ALU.max)
                    eng(mean16_eng).tensor_tensor(
                        st[:, 0, :], mean8[:, 0:N], mean8[:, 1:C],
                        op=ALU.add)
                    m2c16 = winp.tile([128, N], F32, tag="m2c16")
                    eng(m2c16_eng).tensor_tensor(
                        m2c16[:], m2c8[:, 0:N], m2c8[:, 1:C], op=ALU.add)
                    dwin = winp.tile([128, N], F32, tag="dwin")
                    eng(d_eng).tensor_tensor(
                        dwin[:], mean8[:, 0:N], mean8[:, 1:C],
                        op=ALU.subtract)
                    nc.scalar.activation(dwin[:], dwin[:], ACT.Square,
                                         scale=2.0)
                    eng(q_eng).tensor_tensor(
                        m2c16[:], m2c16[:], dwin[:], op=ALU.add)
                    nc.scalar.activation(m2c16[:], m2c16[:], ACT.Relu,
                                         bias=clip_lo[:])
                    nc.scalar.activation(st[:, 2, :], m2c16[:], ACT.Ln,
                                         scale=1.0 / 15.0, bias=ln_bias[:])
                    sts.append(st)

                # projection (contracts d across both halves)
                for eh in range(2):
                    ps = ps_op.tile([128, N], F32, tag="ps_o")
                    k = 0
                    for s3 in range(3):
                        for h in range(2):
                            nc.tensor.matmul(
                                ps[:],
                                wt[:, s3 * 2 + h, eh * 128:(eh + 1) * 128],
                                sts[h][:, s3, :],
                                start=(k == 0), stop=(k == 5))
                            k += 1
                    ob = opool.tile([128, N], F32, tag="ob")
                    nc.scalar.activation(ob[:], ps[:], ACT.Identity,
                                         bias=beff[:, eh:eh + 1], scale=1.0)
                    nc.sync.dma_start(out_ext[b, eh * 128:(eh + 1) * 128, :],
                                      ob[:])

            if rep_ctx is not None:
                rep_ctx.__exit__(None, None, None)

    nc.compile()
    return nc


def _get_nc():
    if "nc" not in _CACHE:
        _CACHE["nc"] = _build()
    return _CACHE["nc"]


def _prep_host(pool_weights, proj_w, proj_b):
    from concourse import mybir
    BF16_NP = mybir.dt.np(mybir.dt.bfloat16)
    FP16_NP = np.float16

    pw = np.asarray(pool_weights, np.float32)
    e = np.exp(pw - pw.max())
    w = (e / e.sum()).astype(np.float32)

    W = np.asarray(proj_w, np.float32)  # [E, D]
    # st0 = mean8[n]+mean8[n+1] = 2*mean16  ->  weight w0/2
    Wcat = np.concatenate(
        [(w[0] / 2.0) * W, w[1] * W, w[2] * W], axis=1)  # [256, 768]
    lhsT = np.ascontiguousarray(Wcat.T)  # [768, 256]
    wt_host = np.ascontiguousarray(
        lhsT.reshape(6, 128, 256).transpose(1, 0, 2)).astype(BF16_NP)
    beff_host = np.ascontiguousarray(
        np.asarray(proj_b, np.float32).reshape(2, 128).T)
    eyeh = np.eye(128, dtype=np.float32).astype(FP16_NP)
    eyeb = np.eye(128, dtype=np.float32).astype(BF16_NP)
    eyen = (-np.eye(128, dtype=np.float32))
    return wt_host, beff_host, eyeh, eyeb, eyen


def _get_runner():
    """Cached jitted SPMD runner (avoids re-tracing the PJRT wrapper on
    every kernel() call).  Mirrors bass2jax.run_bass_via_pjrt."""
    if "runner" in _CACHE:
        return _CACHE["runner"]

    import jax
    from concourse import mybir
    from concourse.bass2jax import (
        _bass_exec_p, install_neuronx_cc_hook, partition_id_tensor)
    from jax.sharding import Mesh, PartitionSpec
    from jax.experimental.shard_map import shard_map

    nc = _get_nc()
    install_neuronx_cc_hook()

    partition_name = (nc.partition_id_tensor.name
                      if nc.partition_id_tensor else None)
    in_names, out_names, out_avals, zero_shapes = [], [], [], []
    for alloc in nc.m.functions[0].allocations:
        if not isinstance(alloc, mybir.MemoryLocationSet):
            continue
        name = alloc.memorylocations[0].name
        if alloc.kind == "ExternalInput":
            if name != partition_name:
                in_names.append(name)
        elif alloc.kind == "ExternalOutput":
            out_names.append(name)
            shape = tuple(alloc.tensor_shape)
            dtype = mybir.dt.np(alloc.dtype)
            out_avals.append(jax.core.ShapedArray(shape, dtype))
            zero_shapes.append((shape, dtype))
    n_params = len(in_names)
    n_outs = len(out_avals)
    all_in = in_names + out_names + ([partition_name] if partition_name else [])

    def _body(*args):
        operands = list(args)
        if partition_name is not None:
            operands.append(partition_id_tensor())
        outs = _bass_exec_p.bind(
            *operands, out_avals=tuple(out_avals), in_names=tuple(all_in),
            out_names=tuple(out_names), lowering_input_output_aliases=(),
            sim_require_finite=True, sim_require_nnan=True, nc=nc)
        return tuple(outs)

    devices = jax.devices()[:N_CORES]
    mesh = Mesh(np.asarray(devices), ("core",))
    in_specs = (PartitionSpec("core"),) * (n_params + n_outs)
    out_specs = (PartitionSpec("core"),) * n_outs
    donate = tuple(range(n_params, n_params + n_outs))
    sharded = jax.jit(
        shard_map(_body, mesh=mesh, in_specs=in_specs, out_specs=out_specs,
                  check_rep=False),
        donate_argnums=donate, keep_unused=True)
    sharding = jax.sharding.NamedSharding(mesh, PartitionSpec("core"))

    def run(in_maps):
        concat_in = [
            np.concatenate(
                [np.asarray(in_maps[c][nm]) for c in range(N_CORES)], axis=0)
            for nm in in_names
        ]
        dev_in = [jax.device_put(a, sharding) for a in concat_in]
        zs = [
            jax.device_put(
                np.zeros((N_CORES * s[0], *s[1:]), dt), sharding)
            for (s, dt) in zero_shapes
        ]
        outs = sharded(*dev_in, *zs)
        return {
            nm: np.asarray(outs[i]).reshape(N_CORES, *out_avals[i].shape)
            for i, nm in enumerate(out_names)
        }

    _CACHE["runner"] = run
    return run


def kernel(x, pool_weights, proj_w, proj_b):
    wt_host, beff_host, eyeh, eyeb, eyen = _prep_host(
        pool_weights, proj_w, proj_b)
    x_f = np.ascontiguousarray(np.asarray(x, np.float32))

    in_maps = [
        {"x": x_f[i * BL:(i + 1) * BL], "wt": wt_host, "beff": beff_host,
         "eyeh": eyeh, "eyeb": eyeb, "eyen": eyen}
        for i in range(N_CORES)
    ]
    res = _get_runner()(in_maps)
    out = res["out"].reshape(B, D, N)
    return np.ascontiguousarray(out.astype(np.float32))
